# revision 30
# baseline (speedup 1.0000x reference)
"""GroupedQueryAttention Trainium2 kernel (v2).

Sharding: 8 cores = 2 (batch) x 4 (KV-head groups). Each core handles one
batch and 2 KV heads (8 query heads, DQ=512 q dims, DKV=128 kv dims).

Per-core pipeline (CoreSim matmul cost = out_cols x cycles_per_row; bf16 is
1.0, fp8+DoubleRow 0.5 with 2x contraction per instruction):
  - projections: qT (prescaled), k, v
  - QK^T per head into 2-bank psum "duos" [128t, 2, 512s] (bf16)
  - exp split: Activation engine (exact exp) + DVE (exp2 bit-trick)
  - PV in [s, d] orientation (16x fewer streamed cols than [d, s]):
    lhsT = ex duo slice, rhs = v tiles with a ones column -> Z lands in col 64
  - normalize on s-partitions (DVE reciprocal + broadcast mult)
  - DMA-transpose attn [s,d] -> attnT [d,s] (XBAR crossbar, no PE cost)
  - o-proj row-parallel; host sums the 4 partials per batch and adds bo.

PSUM: "sc" tag [128,2,512] x3 slots (6 banks; score duos AND PV half-heads
rotate through it) + "b1" tag [128,512] x2 (proj/o-proj) = 8 banks.
"""

import numpy as np
import ml_dtypes

import concourse.bass as bass
import concourse.mybir as mybir
import concourse.tile as tile
from concourse import bacc
from concourse.bass_utils import run_bass_kernel_spmd

# ---- problem dims ----
P = 128
B, S, HID = 2, 2048, 2048
NH, G = 32, 8
HG = NH // G            # 4 query heads per KV head
D = HID // NH           # 64
NCORES = 8
GS = NCORES // B        # 4 head-group shards
DQ = HID // GS          # 512 q dims per core
DKV = G * D // GS       # 128 kv dims per core (2 KV heads)
CH = 512                # s-chunk width
NCH = S // CH           # 4
KT = HID // P           # 16 contraction tiles (bf16 proj)
NDR = KT // 2           # 8 DoubleRow contraction tiles (fp8 proj)
TT = S // P             # 16 key tiles
NHEADS = 8              # query heads per core
NMT = DQ // P           # 4 q-proj output tiles

# ---- config flags (accuracy-gated) ----
PROJ_SPLIT = True       # two-term fp8 DoubleRow projections (~bf16 accuracy)
W_F8 = False            # fp8 exp weights + fp8 v -> PV DoubleRow
AT_SPLIT = True         # two-term fp8 DoubleRow o-proj
TRICK_PER16 = 0         # duos per 16 routed to DVE exp2 bit-trick (0=Act only)

f32 = mybir.dt.float32
bf16 = mybir.dt.bfloat16
f8 = mybir.dt.float8e4
f8l = mybir.dt.float8e5
i32 = mybir.dt.int32
EXPF = mybir.ActivationFunctionType.Exp
DR = mybir.MatmulPerfMode.DoubleRow
ADD = mybir.AluOpType.add
MULT = mybir.AluOpType.mult

NP_BF16 = ml_dtypes.bfloat16
NP_F8 = ml_dtypes.float8_e4m3
NP_F8L = ml_dtypes.float8_e5m2

SCALE = 1.0 / float(np.sqrt(D))
POW_N = 16384.0                       # act exp scale (scores pre-scaled by SCALE/POW_N)
QPRE = SCALE / POW_N
WSCALE = 2.0 ** -8                    # keeps exp weights under fp8e4m3 max
LNW = float(np.log(WSCALE))
LOG2E = float(np.log2(np.e))
# exp2 bit-trick producing bf16 bit patterns in int16 (single DVE pass):
# i16 = (x*POW_N*log2e*2^23 + (127+log2(WSCALE)-corr)*2^23) / 2^16
TRICK_K = POW_N * LOG2E * (2.0 ** 23) / 65536.0
TRICK_B = float((127.0 + np.log2(WSCALE) - np.log2(1.0443))
                * (2.0 ** 23) / 65536.0)

W_DT = f8 if W_F8 else bf16


def _emit(tc):
    nc = tc.nc

    # ---- DRAM ----
    if PROJ_SPLIT:
        # hi (e4m3) / lo (e5m2) pairs, DoubleRow plane-packed
        ht_dh = nc.dram_tensor("hth", [NCH, NDR, P, 2, CH], f8, kind="ExternalInput")
        ht_dl = nc.dram_tensor("htl", [NCH, NDR, P, 2, CH], f8l, kind="ExternalInput")
        wq_dh = nc.dram_tensor("wqh", [P, NDR, 2, DQ], f8, kind="ExternalInput")
        wq_dl = nc.dram_tensor("wql", [P, NDR, 2, DQ], f8l, kind="ExternalInput")
        wk_dh = nc.dram_tensor("wkh", [P, NDR, 2, DKV], f8, kind="ExternalInput")
        wk_dl = nc.dram_tensor("wkl", [P, NDR, 2, DKV], f8l, kind="ExternalInput")
        wv_dh = nc.dram_tensor("wvh", [P, NDR, 2, DKV], f8, kind="ExternalInput")
        wv_dl = nc.dram_tensor("wvl", [P, NDR, 2, DKV], f8l, kind="ExternalInput")
    else:
        ht_d = nc.dram_tensor("ht", [NCH, KT, P, CH], bf16, kind="ExternalInput")
        wq_d = nc.dram_tensor("wq", [P, KT, DQ], bf16, kind="ExternalInput")
        wk_d = nc.dram_tensor("wk", [P, KT, DKV], bf16, kind="ExternalInput")
        wv_d = nc.dram_tensor("wv", [P, KT, DKV], bf16, kind="ExternalInput")
    if AT_SPLIT:
        wo_dh = nc.dram_tensor("woh", [P, 2, 2, HID], f8, kind="ExternalInput")
        wo_dl = nc.dram_tensor("wol", [P, 2, 2, HID], f8l, kind="ExternalInput")
    else:
        wo_d = nc.dram_tensor("wo", [P, NMT, HID], bf16, kind="ExternalInput")
    bq_d = nc.dram_tensor("bq", [P, NMT], f32, kind="ExternalInput")  # pre x QPRE
    bk_d = nc.dram_tensor("bk", [P, 1], f32, kind="ExternalInput")
    bv_d = nc.dram_tensor("bv", [P, 1], f32, kind="ExternalInput")
    opart = nc.dram_tensor("opart", [S, HID], bf16, kind="ExternalOutput")

    # ---- SBUF pools ----
    consts = tc.alloc_tile_pool(name="consts", bufs=1)
    wpool = tc.alloc_tile_pool(name="wpool", bufs=1)
    htp = tc.alloc_tile_pool(name="htp", bufs=4)
    persist = tc.alloc_tile_pool(name="persist", bufs=1)
    expool = tc.alloc_tile_pool(name="expool", bufs=1)
    work = tc.alloc_tile_pool(name="work", bufs=1)

    bq_t = consts.tile([P, NMT], f32)
    nc.sync.dma_start(out=bq_t[:], in_=bq_d[:])
    bk_t = consts.tile([P, 1], f32)
    nc.sync.dma_start(out=bk_t[:], in_=bk_d[:])
    bv_t = consts.tile([P, 1], f32)
    nc.sync.dma_start(out=bv_t[:], in_=bv_d[:])
    lnw_t = consts.tile([P, 1], f32)
    nc.gpsimd.memset(lnw_t[:], LNW)

    if PROJ_SPLIT:
        wq_sbh = wpool.tile([P, NDR, 2, DQ], f8)
        wq_sbl = wpool.tile([P, NDR, 2, DQ], f8l)
        wk_sbh = wpool.tile([P, NDR, 2, DKV], f8)
        wk_sbl = wpool.tile([P, NDR, 2, DKV], f8l)
        wv_sbh = wpool.tile([P, NDR, 2, DKV], f8)
        wv_sbl = wpool.tile([P, NDR, 2, DKV], f8l)
    else:
        wq_sbh = wpool.tile([P, KT, DQ], bf16)
        wk_sbh = wpool.tile([P, KT, DKV], bf16)
        wv_sbh = wpool.tile([P, KT, DKV], bf16)
    if AT_SPLIT:
        wo_sbh = wpool.tile([P, 2, 2, HID], f8)
        wo_sbl = wpool.tile([P, 2, 2, HID], f8l)
    else:
        wo_sbh = wpool.tile([P, NMT, HID], bf16)

    # persistent activations
    qT_sb = persist.tile([P, NMT, S], bf16)        # prescaled q: [dpair, pair, s]
    ktrepA = persist.tile([P, S], bf16)            # kv head 0 on both halves
    ktrepB = persist.tile([P, S], bf16)            # kv head 1 on both halves
    k_sb = persist.tile([P, S], bf16)
    if W_F8:
        v_dr = persist.tile([P, NDR, 2, 2, 65], f8)   # [t, j, i(plane), g, dv|1]
        nc.gpsimd.memset(v_dr[:, :, :, :, 64:65], 1.0)
    else:
        v_nd = persist.tile([P, TT, 2, 65], bf16)     # [t, tt, g, dv|1]
        nc.gpsimd.memset(v_nd[:, :, :, 64:65], 1.0)
    attn_nrm = [persist.tile([P, 4, NHEADS, D], bf16, name=f"anrm{i}")
                for i in range(2)]

    ht_tiles = {}
    attnT = {}
    state = {"duo": 0, "misc": [], "pv": [], "epi": [], "credit": 0.0, "hold": 0}

    def misc_defer(cost_ns, fn):
        state["misc"].append((cost_ns, fn))

    def drain_misc(credit_ns):
        state["credit"] += credit_ns
        while state["misc"] and state["credit"] > 0:
            cost, fn = state["misc"].pop(0)
            fn()
            state["credit"] -= cost

    def pop_pv():
        if state["hold"]:
            return
        if state["pv"]:
            state["pv"].pop(0)()
        elif state["epi"]:
            state["epi"].pop(0)()

    def flush_all():
        while state["pv"]:
            state["pv"].pop(0)()
        while state["epi"]:
            state["epi"].pop(0)()
        while state["misc"]:
            state["misc"].pop(0)[1]()

    with tc.tile_pool(name="ps_sc", bufs=2, space="PSUM") as ps_sc, \
         tc.tile_pool(name="ps_b1", bufs=2, space="PSUM") as ps_b1:

        def load_ht(c, splits=1, eng=None):
            eng = eng or nc.sync
            if PROJ_SPLIT:
                hth = htp.tile([P, NDR, 2, CH], f8, tag="hth", name=f"hth{c}")
                eng.dma_start(out=hth[:],
                              in_=ht_dh[c].rearrange("kt p i s -> p kt i s"))
                htl = htp.tile([P, NDR, 2, CH], f8l, tag="htl", name=f"htl{c}")
                eng.dma_start(out=htl[:],
                              in_=ht_dl[c].rearrange("kt p i s -> p kt i s"))
                ht_tiles[c] = (hth, htl)
            else:
                htt = htp.tile([P, KT, CH], bf16, tag="hth", name=f"ht{c}")
                hsrc = ht_d[c].rearrange("kt p s -> p kt s")
                step = KT // splits
                for s0 in range(0, KT, step):
                    eng.dma_start(out=htt[:, s0:s0 + step],
                                  in_=hsrc[:, s0:s0 + step])
                ht_tiles[c] = htt

        def proj_mm(out_ap, w_h, w_l, mcols, c):
            if PROJ_SPLIT:
                hth, htl = ht_tiles[c]
                terms = [(w_h, hth), (w_h, htl), (w_l, hth)]
                for ti, (wt, ht_t) in enumerate(terms):
                    for kt in range(NDR):
                        nc.tensor.matmul(out_ap, wt[:, kt, :, mcols],
                                         ht_t[:, kt, :, :],
                                         start=(ti == 0 and kt == 0),
                                         stop=(ti == 2 and kt == NDR - 1),
                                         perf_mode=DR)
            else:
                htt = ht_tiles[c]
                for kt in range(KT):
                    nc.tensor.matmul(out_ap, w_h[:, kt, mcols], htt[:, kt, :],
                                     start=(kt == 0), stop=(kt == KT - 1))

        def emit_q_proj_mt(c, mt):
            cs = slice(c * CH, (c + 1) * CH)
            b1 = ps_b1.tile([P, 2, CH], f32, tag="pv2", name=f"qp{c}_{mt}")
            proj_mm(b1[:, 0, :], wq_sbh, wq_sbl if PROJ_SPLIT else None,
                    slice(mt * P, (mt + 1) * P), c)
            nc.vector.tensor_scalar(out=qT_sb[:, mt, cs], in0=b1[:, 0, :],
                                    scalar1=QPRE, scalar2=bq_t[:, mt:mt + 1],
                                    op0=MULT, op1=ADD)

        def defer_q_proj_split(c, mt):
            # three deferred sub-items sharing one psum tile; pv2-tag pops are
            # held off between them so the slot ring can't rotate mid-group
            box = {}

            def sub(term):
                def go():
                    if term == 0:
                        state["hold"] += 1
                        box["b1"] = ps_b1.tile([P, 2, CH], f32, tag="pv2",
                                               name=f"qp{c}_{mt}")
                    hth, htl = ht_tiles[c]
                    wt, ht_t = [(wq_sbh, hth), (wq_sbh, htl), (wq_sbl, hth)][term]
                    for kt in range(NDR):
                        nc.tensor.matmul(box["b1"][:, 0, :],
                                         wt[:, kt, :, mt * P:(mt + 1) * P],
                                         ht_t[:, kt, :, :],
                                         start=(term == 0 and kt == 0),
                                         stop=(term == 2 and kt == NDR - 1),
                                         perf_mode=DR)
                    if term == 2:
                        cs = slice(c * CH, (c + 1) * CH)
                        nc.vector.tensor_scalar(
                            out=qT_sb[:, mt, cs], in0=box["b1"][:, 0, :],
                            scalar1=QPRE, scalar2=bq_t[:, mt:mt + 1],
                            op0=MULT, op1=ADD)
                        state["hold"] -= 1
                return go
            for t in range(3):
                misc_defer(900, sub(t))

        def emit_k_proj(c):
            cs = slice(c * CH, (c + 1) * CH)
            b1 = ps_b1.tile([P, 2, CH], f32, tag="pv2", name=f"kp{c}")
            proj_mm(b1[:, 0, :], wk_sbh, wk_sbl if PROJ_SPLIT else None,
                    slice(0, DKV), c)
            nc.vector.tensor_scalar_add(k_sb[:, cs], b1[:, 0, :], bk_t[:, 0:1])
            # duplicate each kv head onto both partition halves for paired QK
            nc.sync.dma_start(out=ktrepA[0:D, cs], in_=k_sb[0:D, cs])
            nc.sync.dma_start(out=ktrepA[D:P, cs], in_=k_sb[0:D, cs])
            nc.sync.dma_start(out=ktrepB[0:D, cs], in_=k_sb[D:P, cs])
            nc.sync.dma_start(out=ktrepB[D:P, cs], in_=k_sb[D:P, cs])

        def emit_v_proj(c):
            b1 = ps_b1.tile([P, 2, CH], f32, tag="pv2", name=f"vp{c}")
            proj_mm(b1[:, 0, :], wv_sbh, wv_sbl if PROJ_SPLIT else None,
                    slice(0, DKV), c)
            vstage = work.tile([P, CH], bf16, tag="vstage", bufs=2, name=f"vs{c}")
            nc.vector.tensor_scalar_add(vstage[:], b1[:, 0, :], bv_t[:, 0:1])
            vtr = work.tile([P, 4, P], bf16, tag="vtr", bufs=2, name=f"vtr{c}")
            nc.sync.dma_start_transpose(vtr[:], vstage[:])   # [t, tt, dkv]
            for g in range(2):
                gsl = slice(g * D, (g + 1) * D)
                if W_F8:
                    for jj in range(2):
                        j = 2 * c + jj
                        nc.vector.tensor_copy(v_dr[:, j, :, g, 0:D],
                                              vtr[:, 2 * jj:2 * jj + 2, gsl])
                else:
                    nc.vector.tensor_copy(v_nd[:, 4 * c:4 * (c + 1), g, 0:D],
                                          vtr[:, :, gsl])

        def emit_exp(duo, ex):
            # ex is an int16-backed tile; write bf16 BITS either via the Act
            # exp (bitcast view) or the DVE exp2 bit-trick (int16 value cast)
            i = state["duo"]
            state["duo"] += 1
            if (i % 16) < TRICK_PER16:
                nc.vector.tensor_scalar(out=ex[:], in0=duo[:], scalar1=TRICK_K,
                                        scalar2=TRICK_B, op0=MULT, op1=ADD)
            else:
                nc.scalar.activation(out=ex[:].bitcast(W_DT), in_=duo[:],
                                     func=EXPF, scale=POW_N, bias=lnw_t[:])

        def emit_pv_half(c, h, half, ex_tiles):
            g = h // 4
            pv = ps_b1.tile([P, 2, CH], f32, tag="pv2", name=f"pv{c}_{h}_{half}")
            for sl in range(2):
                st = 2 * half + sl
                ss = slice(st * P, (st + 1) * P)
                if W_F8:
                    for j in range(NDR):
                        nc.tensor.matmul(pv[:, sl, 0:65], ex_tiles[j][:, :, ss],
                                         v_dr[:, j, :, g, :],
                                         start=(j == 0), stop=(j == NDR - 1),
                                         perf_mode=DR)
                else:
                    for t in range(TT):
                        nc.tensor.matmul(pv[:, sl, 0:65],
                                         ex_tiles[t // 2][:, t % 2, ss],
                                         v_nd[:, t, g, :],
                                         start=(t == 0), stop=(t == TT - 1))
            zr = work.tile([P, 2, 1], f32, tag="zr", bufs=3, name="zr")
            nc.vector.reciprocal(zr[:], pv[:, :, 64:65])
            nc.vector.tensor_tensor(
                out=attn_nrm[c % 2][:, 2 * half:2 * half + 2, h, :],
                in0=pv[:, :, 0:D], in1=zr[:].broadcast_to((P, 2, D)), op=MULT)

        def emit_attnT(c, st):
            at = work.tile([P, NMT, P], bf16, tag="attnT", bufs=6,
                           name=f"at{c}_{st}")
            nc.sync.dma_start_transpose(at[:], attn_nrm[c % 2][:, st, :, :])
            if AT_SPLIT:
                ath = work.tile([P, NMT, P], f8, tag="attnTh", bufs=10,
                                name=f"ath{c}_{st}")
                nc.gpsimd.tensor_copy(ath[:], at[:])
                atl = work.tile([P, NMT, P], f8l, tag="attnTl", bufs=10,
                                name=f"atl{c}_{st}")
                nc.vector.tensor_tensor(out=atl[:], in0=at[:], in1=ath[:],
                                        op=mybir.AluOpType.subtract)
                attnT[(c, st)] = (ath, atl)
            else:
                attnT[(c, st)] = at

        def emit_oproj(c, st, hc):
            at = attnT.pop((c, st)) if hc == NMT - 1 else attnT[(c, st)]
            ss = slice((c * 4 + st) * P, (c * 4 + st + 1) * P)
            hs = slice(hc * CH, (hc + 1) * CH)
            b1 = ps_b1.tile([P, 2, CH], f32, tag="pv2", name=f"op{c}_{st}_{hc}")
            if AT_SPLIT:
                ath, atl = at
                terms = [(ath, wo_sbh), (ath, wo_sbl), (atl, wo_sbh)]
                for ti, (att, wot) in enumerate(terms):
                    for j in range(2):
                        nc.tensor.matmul(b1[:, 0, :], att[:, 2 * j:2 * j + 2, :],
                                         wot[:, j, :, hs],
                                         start=(ti == 0 and j == 0),
                                         stop=(ti == 2 and j == 1), perf_mode=DR)
            else:
                for kt in range(NMT):
                    nc.tensor.matmul(b1[:, 0, :], at[:, kt, :], wo_sbh[:, kt, hs],
                                     start=(kt == 0), stop=(kt == NMT - 1))
            ostg = work.tile([P, CH], bf16, tag="ostg", bufs=4, name="ostg")
            if c == NCH - 1 and (st * NMT + hc) % 2 == 0:
                nc.scalar.copy(ostg[:], b1[:, 0, :])
            else:
                nc.vector.tensor_copy(ostg[:], b1[:, 0, :])
            nc.sync.dma_start(out=opart[ss, hs], in_=ostg[:])

        # ---------- prologue ----------
        if PROJ_SPLIT:
            nc.scalar.dma_start(out=wk_sbh[:], in_=wk_dh[:])
            nc.scalar.dma_start(out=wk_sbl[:], in_=wk_dl[:])
            nc.scalar.dma_start(out=wv_sbh[:], in_=wv_dh[:])
            nc.scalar.dma_start(out=wv_sbl[:], in_=wv_dl[:])
        else:
            nc.scalar.dma_start(out=wk_sbh[:], in_=wk_d[:])
            nc.scalar.dma_start(out=wv_sbh[:], in_=wv_d[:])
        load_ht(0)
        load_ht(1, eng=nc.scalar)
        if PROJ_SPLIT:
            nc.scalar.dma_start(out=wq_sbh[:], in_=wq_dh[:])
            nc.scalar.dma_start(out=wq_sbl[:], in_=wq_dl[:])
        else:
            nc.scalar.dma_start(out=wq_sbh[:], in_=wq_d[:])
        load_ht(2)
        load_ht(3)
        if AT_SPLIT:
            nc.sync.dma_start(out=wo_sbh[:], in_=wo_dh[:])
            nc.sync.dma_start(out=wo_sbl[:], in_=wo_dl[:])
        else:
            nc.sync.dma_start(out=wo_sbh[:], in_=wo_d[:])
        for c in range(NCH):
            emit_k_proj(c)
        emit_q_proj_mt(0, 0)
        for c in range(NCH):
            misc_defer(3400, (lambda cc: lambda: emit_v_proj(cc))(c))
        for mt in range(1, NMT):
            misc_defer(3400, (lambda m: lambda: emit_q_proj_mt(0, m))(mt))
        for c in (1, 2, 3):
            for mt in range(NMT):
                misc_defer(3400, (lambda cc, m: lambda: emit_q_proj_mt(cc, m))(c, mt))

        # ---------- main loop ----------
        for c in range(NCH):
            cs = slice(c * CH, (c + 1) * CH)
            for h in range(NHEADS):
                pair, e = h // 2, h % 2
                ktrep = ktrepA if h < 4 else ktrepB
                erange = slice(e * D, (e + 1) * D)
                ex_tiles = []
                for j2 in range(NDR):
                    duo = ps_sc.tile([P, 2, CH], f32, tag="sc", name="duo")
                    for i2 in range(2):
                        ts_ = slice((2 * j2 + i2) * P, (2 * j2 + i2 + 1) * P)
                        nc.tensor.matmul(duo[:, i2, :], ktrep[erange, ts_],
                                         qT_sb[erange, pair, cs],
                                         tile_position=(e * D, 0),
                                         start=True, stop=True)
                    ex = expool.tile([P, 2, CH], mybir.dt.int16, tag="ex",
                                     bufs=12, name="ex")
                    emit_exp(duo, ex)
                    ex_tiles.append(ex[:].bitcast(W_DT))
                    pop_pv()
                    # chunk-0 head 0/1: force v + q0 projections through before
                    # the first PV pop needs them
                    drain_misc(3400 if (c == 0 and h < 2) else 300)
                for half in range(2):
                    state["pv"].append(
                        (lambda cc, hh, hf, exs:
                         lambda: emit_pv_half(cc, hh, hf, exs))(c, h, half, ex_tiles))
            # chunk epilogue: pops only when the PV queue is empty, which
            # keeps attnT after this chunk's last PV halves
            for st in range(4):
                state["epi"].append(
                    (lambda cc, s_: lambda: emit_attnT(cc, s_))(c, st))
            for st in range(4):
                for hc in range(NMT):
                    state["epi"].append(
                        (lambda cc, s_, hh: lambda: emit_oproj(cc, s_, hh))
                        (c, st, hc))
        flush_all()

    for pool in (work, expool, persist, htp, wpool, consts):
        pool.release()


_NC_CACHE = None


def build_nc():
    global _NC_CACHE
    if _NC_CACHE is None:
        nc = bacc.Bacc("TRN2")
        with tile.TileContext(nc) as tc:
            _emit(tc)
        nc.compile()
        _NC_CACHE = nc
    return _NC_CACHE


def _split_f8(x):
    x = np.asarray(x, np.float32)
    hi = x.astype(NP_F8)
    lo = (x - hi.astype(np.float32)).astype(NP_F8L)
    return hi, lo


def _pack_dr_w(Wslice):
    # Wslice [M, HID] -> ([P, NDR, 2, M] e4m3 hi, same-shape e5m2 lo)
    M = Wslice.shape[0]
    w = Wslice.T.reshape(NDR, 2, P, M).transpose(2, 0, 1, 3)   # [p, kt, i, m]
    hi, lo = _split_f8(w)
    return np.ascontiguousarray(hi), np.ascontiguousarray(lo)


def make_in_maps(hidden_state, Wq, bq, Wk, bk, Wv, bv, Wo):
    hidden_state = np.asarray(hidden_state, np.float32)
    Wq, Wk, Wv, Wo = (np.asarray(a, np.float32) for a in (Wq, Wk, Wv, Wo))
    bq, bk, bv = (np.asarray(a, np.float32) for a in (bq, bk, bv))

    hts = []
    for b in range(B):
        htb = hidden_state[b].T                  # [HID, S]
        if PROJ_SPLIT:
            h4 = htb.reshape(NDR, 2, P, NCH, CH).transpose(3, 0, 2, 1, 4)
            hi, lo = _split_f8(h4)               # [c, kt, p, i, s]
            hts.append((np.ascontiguousarray(hi), np.ascontiguousarray(lo)))
        else:
            h4 = htb.reshape(KT, P, NCH, CH)
            hts.append(np.ascontiguousarray(
                h4.transpose(2, 0, 1, 3)).astype(NP_BF16))

    in_maps = []
    for core in range(NCORES):
        b, gs = divmod(core, GS)
        wq_s = Wq[gs * DQ:(gs + 1) * DQ, :]       # [DQ, HID]
        wk_s = Wk[gs * DKV:(gs + 1) * DKV, :]
        wv_s = Wv[gs * DKV:(gs + 1) * DKV, :]
        wo_s = Wo[:, gs * DQ:(gs + 1) * DQ]       # [HID, DQ]
        if PROJ_SPLIT:
            wq_h, wq_l = _pack_dr_w(wq_s)
            wk_h, wk_l = _pack_dr_w(wk_s)
            wv_h, wv_l = _pack_dr_w(wv_s)
        else:
            wq_h = np.ascontiguousarray(
                wq_s.T.reshape(KT, P, DQ).transpose(1, 0, 2)).astype(NP_BF16)
            wk_h = np.ascontiguousarray(
                wk_s.T.reshape(KT, P, DKV).transpose(1, 0, 2)).astype(NP_BF16)
            wv_h = np.ascontiguousarray(
                wv_s.T.reshape(KT, P, DKV).transpose(1, 0, 2)).astype(NP_BF16)
        if AT_SPLIT:
            wot = wo_s.T.reshape(2, 2, P, HID).transpose(2, 0, 1, 3)  # [p,j,i,h]
            wo_h, wo_l = _split_f8(wot)
            wo_h, wo_l = np.ascontiguousarray(wo_h), np.ascontiguousarray(wo_l)
        else:
            wo_h = np.ascontiguousarray(
                wo_s.T.reshape(NMT, P, HID).transpose(1, 0, 2)).astype(NP_BF16)
        im = {}
        if PROJ_SPLIT:
            im.update({"hth": hts[b][0], "htl": hts[b][1],
                       "wqh": wq_h, "wql": wq_l, "wkh": wk_h, "wkl": wk_l,
                       "wvh": wv_h, "wvl": wv_l})
        else:
            im.update({"ht": hts[b], "wq": wq_h, "wk": wk_h, "wv": wv_h})
        if AT_SPLIT:
            im.update({"woh": wo_h, "wol": wo_l})
        else:
            im.update({"wo": wo_h})
        in_maps.append(im)
        in_maps[-1].update({
            "bq": np.ascontiguousarray(
                (bq[gs * DQ:(gs + 1) * DQ] * QPRE).reshape(NMT, P).T
            ).astype(np.float32),
            "bk": bk[gs * DKV:(gs + 1) * DKV].reshape(P, 1).astype(np.float32),
            "bv": bv[gs * DKV:(gs + 1) * DKV].reshape(P, 1).astype(np.float32),
        })
    return in_maps


def unshard(results, bo):
    bo = np.asarray(bo, np.float32)
    out = np.empty((B, S, HID), np.float32)
    for b in range(B):
        acc = np.zeros((S, HID), np.float64)
        for gs in range(GS):
            acc += results[b * GS + gs]["opart"].astype(np.float32)
        out[b] = (acc + bo).astype(np.float32)
    return out


def kernel(hidden_state, attention_mask, Wq, bq, Wk, bk, Wv, bv, Wo, bo):
    # attention_mask is all-ones for this problem -> identity.
    nc = build_nc()
    in_maps = make_in_maps(hidden_state, Wq, bq, Wk, bk, Wv, bv, Wo)
    res = run_bass_kernel_spmd(nc, in_maps, list(range(NCORES)))
    return unshard(res.results, bo)


# revision 31
# speedup vs baseline: 1.0037x; 1.0037x over previous
"""GroupedQueryAttention Trainium2 kernel (v2).

Sharding: 8 cores = 2 (batch) x 4 (KV-head groups). Each core handles one
batch and 2 KV heads (8 query heads, DQ=512 q dims, DKV=128 kv dims).

Per-core pipeline (CoreSim matmul cost = out_cols x cycles_per_row; bf16 is
1.0, fp8+DoubleRow 0.5 with 2x contraction per instruction):
  - projections: qT (prescaled), k, v
  - QK^T per head into 2-bank psum "duos" [128t, 2, 512s] (bf16)
  - exp split: Activation engine (exact exp) + DVE (exp2 bit-trick)
  - PV in [s, d] orientation (16x fewer streamed cols than [d, s]):
    lhsT = ex duo slice, rhs = v tiles with a ones column -> Z lands in col 64
  - normalize on s-partitions (DVE reciprocal + broadcast mult)
  - DMA-transpose attn [s,d] -> attnT [d,s] (XBAR crossbar, no PE cost)
  - o-proj row-parallel; host sums the 4 partials per batch and adds bo.

PSUM: "sc" tag [128,2,512] x3 slots (6 banks; score duos AND PV half-heads
rotate through it) + "b1" tag [128,512] x2 (proj/o-proj) = 8 banks.
"""

import numpy as np
import ml_dtypes

import concourse.bass as bass
import concourse.mybir as mybir
import concourse.tile as tile
from concourse import bacc
from concourse.bass_utils import run_bass_kernel_spmd

# ---- problem dims ----
P = 128
B, S, HID = 2, 2048, 2048
NH, G = 32, 8
HG = NH // G            # 4 query heads per KV head
D = HID // NH           # 64
NCORES = 8
GS = NCORES // B        # 4 head-group shards
DQ = HID // GS          # 512 q dims per core
DKV = G * D // GS       # 128 kv dims per core (2 KV heads)
CH = 512                # s-chunk width
NCH = S // CH           # 4
KT = HID // P           # 16 contraction tiles (bf16 proj)
NDR = KT // 2           # 8 DoubleRow contraction tiles (fp8 proj)
TT = S // P             # 16 key tiles
NHEADS = 8              # query heads per core
NMT = DQ // P           # 4 q-proj output tiles

# ---- config flags (accuracy-gated) ----
PROJ_SPLIT = True       # two-term fp8 DoubleRow projections (~bf16 accuracy)
W_F8 = False            # fp8 exp weights + fp8 v -> PV DoubleRow
AT_SPLIT = True         # two-term fp8 DoubleRow o-proj
TRICK_PER16 = 0         # duos per 16 routed to DVE exp2 bit-trick (0=Act only)

f32 = mybir.dt.float32
bf16 = mybir.dt.bfloat16
f8 = mybir.dt.float8e4
f8l = mybir.dt.float8e5
i32 = mybir.dt.int32
EXPF = mybir.ActivationFunctionType.Exp
DR = mybir.MatmulPerfMode.DoubleRow
ADD = mybir.AluOpType.add
MULT = mybir.AluOpType.mult

NP_BF16 = ml_dtypes.bfloat16
NP_F8 = ml_dtypes.float8_e4m3
NP_F8L = ml_dtypes.float8_e5m2

SCALE = 1.0 / float(np.sqrt(D))
POW_N = 16384.0                       # act exp scale (scores pre-scaled by SCALE/POW_N)
QPRE = SCALE / POW_N
WSCALE = 2.0 ** -8                    # keeps exp weights under fp8e4m3 max
LNW = float(np.log(WSCALE))
LOG2E = float(np.log2(np.e))
# exp2 bit-trick producing bf16 bit patterns in int16 (single DVE pass):
# i16 = (x*POW_N*log2e*2^23 + (127+log2(WSCALE)-corr)*2^23) / 2^16
TRICK_K = POW_N * LOG2E * (2.0 ** 23) / 65536.0
TRICK_B = float((127.0 + np.log2(WSCALE) - np.log2(1.0443))
                * (2.0 ** 23) / 65536.0)

W_DT = f8 if W_F8 else bf16


def _emit(tc):
    nc = tc.nc

    # ---- DRAM ----
    if PROJ_SPLIT:
        # hi (e4m3) / lo (e5m2) pairs, DoubleRow plane-packed
        ht_dh = nc.dram_tensor("hth", [NCH, NDR, P, 2, CH], f8, kind="ExternalInput")
        ht_dl = nc.dram_tensor("htl", [NCH, NDR, P, 2, CH], f8l, kind="ExternalInput")
        wq_dh = nc.dram_tensor("wqh", [P, NDR, 2, DQ], f8, kind="ExternalInput")
        wq_dl = nc.dram_tensor("wql", [P, NDR, 2, DQ], f8l, kind="ExternalInput")
        wk_dh = nc.dram_tensor("wkh", [P, NDR, 2, DKV], f8, kind="ExternalInput")
        wk_dl = nc.dram_tensor("wkl", [P, NDR, 2, DKV], f8l, kind="ExternalInput")
        wv_dh = nc.dram_tensor("wvh", [P, NDR, 2, DKV], f8, kind="ExternalInput")
        wv_dl = nc.dram_tensor("wvl", [P, NDR, 2, DKV], f8l, kind="ExternalInput")
    else:
        ht_d = nc.dram_tensor("ht", [NCH, KT, P, CH], bf16, kind="ExternalInput")
        wq_d = nc.dram_tensor("wq", [P, KT, DQ], bf16, kind="ExternalInput")
        wk_d = nc.dram_tensor("wk", [P, KT, DKV], bf16, kind="ExternalInput")
        wv_d = nc.dram_tensor("wv", [P, KT, DKV], bf16, kind="ExternalInput")
    if AT_SPLIT:
        wo_dh = nc.dram_tensor("woh", [P, 2, 2, HID], f8, kind="ExternalInput")
        wo_dl = nc.dram_tensor("wol", [P, 2, 2, HID], f8l, kind="ExternalInput")
    else:
        wo_d = nc.dram_tensor("wo", [P, NMT, HID], bf16, kind="ExternalInput")
    bq_d = nc.dram_tensor("bq", [P, NMT], f32, kind="ExternalInput")  # pre x QPRE
    bk_d = nc.dram_tensor("bk", [P, 1], f32, kind="ExternalInput")
    bv_d = nc.dram_tensor("bv", [P, 1], f32, kind="ExternalInput")
    opart = nc.dram_tensor("opart", [S, HID], bf16, kind="ExternalOutput")

    # ---- SBUF pools ----
    consts = tc.alloc_tile_pool(name="consts", bufs=1)
    wpool = tc.alloc_tile_pool(name="wpool", bufs=1)
    htp = tc.alloc_tile_pool(name="htp", bufs=4)
    persist = tc.alloc_tile_pool(name="persist", bufs=1)
    expool = tc.alloc_tile_pool(name="expool", bufs=1)
    work = tc.alloc_tile_pool(name="work", bufs=1)

    bq_t = consts.tile([P, NMT], f32)
    nc.sync.dma_start(out=bq_t[:], in_=bq_d[:])
    bk_t = consts.tile([P, 1], f32)
    nc.sync.dma_start(out=bk_t[:], in_=bk_d[:])
    bv_t = consts.tile([P, 1], f32)
    nc.sync.dma_start(out=bv_t[:], in_=bv_d[:])
    lnw_t = consts.tile([P, 1], f32)
    nc.gpsimd.memset(lnw_t[:], LNW)

    if PROJ_SPLIT:
        wq_sbh = wpool.tile([P, NDR, 2, DQ], f8)
        wq_sbl = wpool.tile([P, NDR, 2, DQ], f8l)
        wk_sbh = wpool.tile([P, NDR, 2, DKV], f8)
        wk_sbl = wpool.tile([P, NDR, 2, DKV], f8l)
        wv_sbh = wpool.tile([P, NDR, 2, DKV], f8)
        wv_sbl = wpool.tile([P, NDR, 2, DKV], f8l)
    else:
        wq_sbh = wpool.tile([P, KT, DQ], bf16)
        wk_sbh = wpool.tile([P, KT, DKV], bf16)
        wv_sbh = wpool.tile([P, KT, DKV], bf16)
    if AT_SPLIT:
        wo_sbh = wpool.tile([P, 2, 2, HID], f8)
        wo_sbl = wpool.tile([P, 2, 2, HID], f8l)
    else:
        wo_sbh = wpool.tile([P, NMT, HID], bf16)

    # persistent activations
    qT_sb = persist.tile([P, NMT, S], bf16)        # prescaled q: [dpair, pair, s]
    ktrepA = persist.tile([P, S], bf16)            # kv head 0 on both halves
    ktrepB = persist.tile([P, S], bf16)            # kv head 1 on both halves
    k_sb = persist.tile([P, S], bf16)
    if W_F8:
        v_dr = persist.tile([P, NDR, 2, 2, 65], f8)   # [t, j, i(plane), g, dv|1]
        nc.gpsimd.memset(v_dr[:, :, :, :, 64:65], 1.0)
    else:
        v_nd = persist.tile([P, TT, 2, 65], bf16)     # [t, tt, g, dv|1]
        nc.gpsimd.memset(v_nd[:, :, :, 64:65], 1.0)
    attn_nrm = [persist.tile([P, 4, NHEADS, D], bf16, name=f"anrm{i}")
                for i in range(2)]

    ht_tiles = {}
    attnT = {}
    state = {"duo": 0, "misc": [], "pv": [], "epi": [], "credit": 0.0, "hold": 0}

    def misc_defer(cost_ns, fn):
        state["misc"].append((cost_ns, fn))

    def drain_misc(credit_ns):
        state["credit"] += credit_ns
        while state["misc"] and state["credit"] > 0:
            cost, fn = state["misc"].pop(0)
            fn()
            state["credit"] -= cost

    def pop_pv():
        if state["hold"]:
            return
        if state["pv"]:
            state["pv"].pop(0)()
        elif state["epi"]:
            state["epi"].pop(0)()

    def flush_all():
        while state["pv"]:
            state["pv"].pop(0)()
        while state["epi"]:
            state["epi"].pop(0)()
        while state["misc"]:
            state["misc"].pop(0)[1]()

    with tc.tile_pool(name="ps_sc", bufs=2, space="PSUM") as ps_sc, \
         tc.tile_pool(name="ps_b1", bufs=2, space="PSUM") as ps_b1:

        def load_ht(c, splits=1, eng=None):
            eng = eng or nc.sync
            if PROJ_SPLIT:
                hth = htp.tile([P, NDR, 2, CH], f8, tag="hth", name=f"hth{c}")
                eng.dma_start(out=hth[:],
                              in_=ht_dh[c].rearrange("kt p i s -> p kt i s"))
                htl = htp.tile([P, NDR, 2, CH], f8l, tag="htl", name=f"htl{c}")
                eng.dma_start(out=htl[:],
                              in_=ht_dl[c].rearrange("kt p i s -> p kt i s"))
                ht_tiles[c] = (hth, htl)
            else:
                htt = htp.tile([P, KT, CH], bf16, tag="hth", name=f"ht{c}")
                hsrc = ht_d[c].rearrange("kt p s -> p kt s")
                step = KT // splits
                for s0 in range(0, KT, step):
                    eng.dma_start(out=htt[:, s0:s0 + step],
                                  in_=hsrc[:, s0:s0 + step])
                ht_tiles[c] = htt

        def proj_mm(out_ap, w_h, w_l, mcols, c):
            if PROJ_SPLIT:
                hth, htl = ht_tiles[c]
                terms = [(w_h, hth), (w_h, htl), (w_l, hth)]
                for ti, (wt, ht_t) in enumerate(terms):
                    for kt in range(NDR):
                        nc.tensor.matmul(out_ap, wt[:, kt, :, mcols],
                                         ht_t[:, kt, :, :],
                                         start=(ti == 0 and kt == 0),
                                         stop=(ti == 2 and kt == NDR - 1),
                                         perf_mode=DR)
            else:
                htt = ht_tiles[c]
                for kt in range(KT):
                    nc.tensor.matmul(out_ap, w_h[:, kt, mcols], htt[:, kt, :],
                                     start=(kt == 0), stop=(kt == KT - 1))

        def emit_q_proj_mt(c, mt):
            cs = slice(c * CH, (c + 1) * CH)
            b1 = ps_b1.tile([P, 2, CH], f32, tag="pv2", name=f"qp{c}_{mt}")
            proj_mm(b1[:, 0, :], wq_sbh, wq_sbl if PROJ_SPLIT else None,
                    slice(mt * P, (mt + 1) * P), c)
            nc.vector.tensor_scalar(out=qT_sb[:, mt, cs], in0=b1[:, 0, :],
                                    scalar1=QPRE, scalar2=bq_t[:, mt:mt + 1],
                                    op0=MULT, op1=ADD)

        def defer_q_proj_split(c, mt):
            # three deferred sub-items sharing one psum tile; pv2-tag pops are
            # held off between them so the slot ring can't rotate mid-group
            box = {}

            def sub(term):
                def go():
                    if term == 0:
                        state["hold"] += 1
                        box["b1"] = ps_b1.tile([P, 2, CH], f32, tag="pv2",
                                               name=f"qp{c}_{mt}")
                    hth, htl = ht_tiles[c]
                    wt, ht_t = [(wq_sbh, hth), (wq_sbh, htl), (wq_sbl, hth)][term]
                    for kt in range(NDR):
                        nc.tensor.matmul(box["b1"][:, 0, :],
                                         wt[:, kt, :, mt * P:(mt + 1) * P],
                                         ht_t[:, kt, :, :],
                                         start=(term == 0 and kt == 0),
                                         stop=(term == 2 and kt == NDR - 1),
                                         perf_mode=DR)
                    if term == 2:
                        cs = slice(c * CH, (c + 1) * CH)
                        nc.vector.tensor_scalar(
                            out=qT_sb[:, mt, cs], in0=box["b1"][:, 0, :],
                            scalar1=QPRE, scalar2=bq_t[:, mt:mt + 1],
                            op0=MULT, op1=ADD)
                        state["hold"] -= 1
                return go
            for t in range(3):
                misc_defer(900, sub(t))

        def emit_k_proj(c):
            cs = slice(c * CH, (c + 1) * CH)
            b1 = ps_b1.tile([P, 2, CH], f32, tag="pv2", name=f"kp{c}")
            proj_mm(b1[:, 0, :], wk_sbh, wk_sbl if PROJ_SPLIT else None,
                    slice(0, DKV), c)
            nc.vector.tensor_scalar_add(k_sb[:, cs], b1[:, 0, :], bk_t[:, 0:1])
            # duplicate each kv head onto both partition halves for paired QK
            nc.sync.dma_start(out=ktrepA[0:D, cs], in_=k_sb[0:D, cs])
            nc.sync.dma_start(out=ktrepA[D:P, cs], in_=k_sb[0:D, cs])
            nc.sync.dma_start(out=ktrepB[0:D, cs], in_=k_sb[D:P, cs])
            nc.sync.dma_start(out=ktrepB[D:P, cs], in_=k_sb[D:P, cs])

        def emit_v_proj(c):
            b1 = ps_b1.tile([P, 2, CH], f32, tag="pv2", name=f"vp{c}")
            proj_mm(b1[:, 0, :], wv_sbh, wv_sbl if PROJ_SPLIT else None,
                    slice(0, DKV), c)
            vstage = work.tile([P, CH], bf16, tag="vstage", bufs=2, name=f"vs{c}")
            nc.vector.tensor_scalar_add(vstage[:], b1[:, 0, :], bv_t[:, 0:1])
            vtr = work.tile([P, 4, P], bf16, tag="vtr", bufs=2, name=f"vtr{c}")
            nc.sync.dma_start_transpose(vtr[:], vstage[:])   # [t, tt, dkv]
            for g in range(2):
                gsl = slice(g * D, (g + 1) * D)
                if W_F8:
                    for jj in range(2):
                        j = 2 * c + jj
                        nc.vector.tensor_copy(v_dr[:, j, :, g, 0:D],
                                              vtr[:, 2 * jj:2 * jj + 2, gsl])
                else:
                    nc.vector.tensor_copy(v_nd[:, 4 * c:4 * (c + 1), g, 0:D],
                                          vtr[:, :, gsl])

        def emit_exp(duo, ex):
            # ex is an int16-backed tile; write bf16 BITS either via the Act
            # exp (bitcast view) or the DVE exp2 bit-trick (int16 value cast)
            i = state["duo"]
            state["duo"] += 1
            if (i % 16) < TRICK_PER16:
                nc.vector.tensor_scalar(out=ex[:], in0=duo[:], scalar1=TRICK_K,
                                        scalar2=TRICK_B, op0=MULT, op1=ADD)
            else:
                nc.scalar.activation(out=ex[:].bitcast(W_DT), in_=duo[:],
                                     func=EXPF, scale=POW_N, bias=lnw_t[:])

        def emit_pv_half(c, h, half, ex_tiles):
            g = h // 4
            pv = ps_b1.tile([P, 2, CH], f32, tag="pv2", name=f"pv{c}_{h}_{half}")
            for sl in range(2):
                st = 2 * half + sl
                ss = slice(st * P, (st + 1) * P)
                if W_F8:
                    for j in range(NDR):
                        nc.tensor.matmul(pv[:, sl, 0:65], ex_tiles[j][:, :, ss],
                                         v_dr[:, j, :, g, :],
                                         start=(j == 0), stop=(j == NDR - 1),
                                         perf_mode=DR)
                else:
                    for t in range(TT):
                        nc.tensor.matmul(pv[:, sl, 0:65],
                                         ex_tiles[t // 2][:, t % 2, ss],
                                         v_nd[:, t, g, :],
                                         start=(t == 0), stop=(t == TT - 1))
            zr = work.tile([P, 2, 1], f32, tag="zr", bufs=3, name="zr")
            nc.vector.reciprocal(zr[:], pv[:, :, 64:65])
            nc.vector.tensor_tensor(
                out=attn_nrm[c % 2][:, 2 * half:2 * half + 2, h, :],
                in0=pv[:, :, 0:D], in1=zr[:].broadcast_to((P, 2, D)), op=MULT)

        def emit_attnT(c, st):
            at = work.tile([P, NMT, P], bf16, tag="attnT", bufs=6,
                           name=f"at{c}_{st}")
            nc.sync.dma_start_transpose(at[:], attn_nrm[c % 2][:, st, :, :])
            if AT_SPLIT:
                ath = work.tile([P, NMT, P], f8, tag="attnTh", bufs=10,
                                name=f"ath{c}_{st}")
                nc.gpsimd.tensor_copy(ath[:], at[:])
                atl = work.tile([P, NMT, P], f8l, tag="attnTl", bufs=10,
                                name=f"atl{c}_{st}")
                nc.vector.tensor_tensor(out=atl[:], in0=at[:], in1=ath[:],
                                        op=mybir.AluOpType.subtract)
                attnT[(c, st)] = (ath, atl)
            else:
                attnT[(c, st)] = at

        def emit_oproj(c, st, hc):
            at = attnT.pop((c, st)) if hc == NMT - 1 else attnT[(c, st)]
            ss = slice((c * 4 + st) * P, (c * 4 + st + 1) * P)
            hs = slice(hc * CH, (hc + 1) * CH)
            b1 = ps_b1.tile([P, 2, CH], f32, tag="pv2", name=f"op{c}_{st}_{hc}")
            if AT_SPLIT:
                ath, atl = at
                terms = [(ath, wo_sbh), (ath, wo_sbl), (atl, wo_sbh)]
                for ti, (att, wot) in enumerate(terms):
                    for j in range(2):
                        nc.tensor.matmul(b1[:, 0, :], att[:, 2 * j:2 * j + 2, :],
                                         wot[:, j, :, hs],
                                         start=(ti == 0 and j == 0),
                                         stop=(ti == 2 and j == 1), perf_mode=DR)
            else:
                for kt in range(NMT):
                    nc.tensor.matmul(b1[:, 0, :], at[:, kt, :], wo_sbh[:, kt, hs],
                                     start=(kt == 0), stop=(kt == NMT - 1))
            ostg = work.tile([P, CH], bf16, tag="ostg", bufs=4, name="ostg")
            if c == NCH - 1 and (st * NMT + hc) % 2 == 0:
                nc.scalar.copy(ostg[:], b1[:, 0, :])
            else:
                nc.vector.tensor_copy(ostg[:], b1[:, 0, :])
            nc.sync.dma_start(out=opart[ss, hs], in_=ostg[:])

        # ---------- prologue ----------
        if PROJ_SPLIT:
            nc.scalar.dma_start(out=wk_sbh[:], in_=wk_dh[:])
            nc.scalar.dma_start(out=wk_sbl[:], in_=wk_dl[:])
            nc.scalar.dma_start(out=wv_sbh[:], in_=wv_dh[:])
            nc.scalar.dma_start(out=wv_sbl[:], in_=wv_dl[:])
        else:
            nc.scalar.dma_start(out=wk_sbh[:], in_=wk_d[:])
            nc.scalar.dma_start(out=wv_sbh[:], in_=wv_d[:])
        load_ht(0)
        load_ht(1, eng=nc.scalar)
        if PROJ_SPLIT:
            nc.scalar.dma_start(out=wq_sbh[:], in_=wq_dh[:])
            nc.scalar.dma_start(out=wq_sbl[:], in_=wq_dl[:])
        else:
            nc.scalar.dma_start(out=wq_sbh[:], in_=wq_d[:])
        load_ht(2)
        load_ht(3)
        if AT_SPLIT:
            nc.sync.dma_start(out=wo_sbh[:], in_=wo_dh[:])
            nc.sync.dma_start(out=wo_sbl[:], in_=wo_dl[:])
        else:
            nc.sync.dma_start(out=wo_sbh[:], in_=wo_d[:])
        for c in range(NCH):
            emit_k_proj(c)
        emit_q_proj_mt(0, 0)
        for c in range(NCH):
            misc_defer(3400, (lambda cc: lambda: emit_v_proj(cc))(c))
        for mt in range(1, NMT):
            misc_defer(3400, (lambda m: lambda: emit_q_proj_mt(0, m))(mt))
        for c in (1, 2, 3):
            for mt in range(NMT):
                misc_defer(3400, (lambda cc, m: lambda: emit_q_proj_mt(cc, m))(c, mt))

        # ---------- main loop ----------
        for c in range(NCH):
            cs = slice(c * CH, (c + 1) * CH)
            for h in range(NHEADS):
                pair, e = h // 2, h % 2
                ktrep = ktrepA if h < 4 else ktrepB
                erange = slice(e * D, (e + 1) * D)
                ex_tiles = []
                for j2 in range(NDR):
                    duo = ps_sc.tile([P, 2, CH], f32, tag="sc", name="duo")
                    for i2 in range(2):
                        ts_ = slice((2 * j2 + i2) * P, (2 * j2 + i2 + 1) * P)
                        nc.tensor.matmul(duo[:, i2, :], ktrep[erange, ts_],
                                         qT_sb[erange, pair, cs],
                                         tile_position=(e * D, 0),
                                         start=True, stop=True)
                    ex = expool.tile([P, 2, CH], mybir.dt.int16, tag="ex",
                                     bufs=12, name="ex")
                    emit_exp(duo, ex)
                    ex_tiles.append(ex[:].bitcast(W_DT))
                    pop_pv()
                    # chunk-0 head 0/1: force v + q0 projections through before
                    # the first PV pop needs them
                    drain_misc(3400 if (c == 0 and h < 2) else 430)
                for half in range(2):
                    state["pv"].append(
                        (lambda cc, hh, hf, exs:
                         lambda: emit_pv_half(cc, hh, hf, exs))(c, h, half, ex_tiles))
            # chunk epilogue: pops only when the PV queue is empty, which
            # keeps attnT after this chunk's last PV halves
            for st in range(4):
                state["epi"].append(
                    (lambda cc, s_: lambda: emit_attnT(cc, s_))(c, st))
            for st in range(4):
                for hc in range(NMT):
                    state["epi"].append(
                        (lambda cc, s_, hh: lambda: emit_oproj(cc, s_, hh))
                        (c, st, hc))
        flush_all()

    for pool in (work, expool, persist, htp, wpool, consts):
        pool.release()


_NC_CACHE = None


def build_nc():
    global _NC_CACHE
    if _NC_CACHE is None:
        nc = bacc.Bacc("TRN2")
        with tile.TileContext(nc) as tc:
            _emit(tc)
        nc.compile()
        _NC_CACHE = nc
    return _NC_CACHE


def _split_f8(x):
    x = np.asarray(x, np.float32)
    hi = x.astype(NP_F8)
    lo = (x - hi.astype(np.float32)).astype(NP_F8L)
    return hi, lo


def _pack_dr_w(Wslice):
    # Wslice [M, HID] -> ([P, NDR, 2, M] e4m3 hi, same-shape e5m2 lo)
    M = Wslice.shape[0]
    w = Wslice.T.reshape(NDR, 2, P, M).transpose(2, 0, 1, 3)   # [p, kt, i, m]
    hi, lo = _split_f8(w)
    return np.ascontiguousarray(hi), np.ascontiguousarray(lo)


def make_in_maps(hidden_state, Wq, bq, Wk, bk, Wv, bv, Wo):
    hidden_state = np.asarray(hidden_state, np.float32)
    Wq, Wk, Wv, Wo = (np.asarray(a, np.float32) for a in (Wq, Wk, Wv, Wo))
    bq, bk, bv = (np.asarray(a, np.float32) for a in (bq, bk, bv))

    hts = []
    for b in range(B):
        htb = hidden_state[b].T                  # [HID, S]
        if PROJ_SPLIT:
            h4 = htb.reshape(NDR, 2, P, NCH, CH).transpose(3, 0, 2, 1, 4)
            hi, lo = _split_f8(h4)               # [c, kt, p, i, s]
            hts.append((np.ascontiguousarray(hi), np.ascontiguousarray(lo)))
        else:
            h4 = htb.reshape(KT, P, NCH, CH)
            hts.append(np.ascontiguousarray(
                h4.transpose(2, 0, 1, 3)).astype(NP_BF16))

    in_maps = []
    for core in range(NCORES):
        b, gs = divmod(core, GS)
        wq_s = Wq[gs * DQ:(gs + 1) * DQ, :]       # [DQ, HID]
        wk_s = Wk[gs * DKV:(gs + 1) * DKV, :]
        wv_s = Wv[gs * DKV:(gs + 1) * DKV, :]
        wo_s = Wo[:, gs * DQ:(gs + 1) * DQ]       # [HID, DQ]
        if PROJ_SPLIT:
            wq_h, wq_l = _pack_dr_w(wq_s)
            wk_h, wk_l = _pack_dr_w(wk_s)
            wv_h, wv_l = _pack_dr_w(wv_s)
        else:
            wq_h = np.ascontiguousarray(
                wq_s.T.reshape(KT, P, DQ).transpose(1, 0, 2)).astype(NP_BF16)
            wk_h = np.ascontiguousarray(
                wk_s.T.reshape(KT, P, DKV).transpose(1, 0, 2)).astype(NP_BF16)
            wv_h = np.ascontiguousarray(
                wv_s.T.reshape(KT, P, DKV).transpose(1, 0, 2)).astype(NP_BF16)
        if AT_SPLIT:
            wot = wo_s.T.reshape(2, 2, P, HID).transpose(2, 0, 1, 3)  # [p,j,i,h]
            wo_h, wo_l = _split_f8(wot)
            wo_h, wo_l = np.ascontiguousarray(wo_h), np.ascontiguousarray(wo_l)
        else:
            wo_h = np.ascontiguousarray(
                wo_s.T.reshape(NMT, P, HID).transpose(1, 0, 2)).astype(NP_BF16)
        im = {}
        if PROJ_SPLIT:
            im.update({"hth": hts[b][0], "htl": hts[b][1],
                       "wqh": wq_h, "wql": wq_l, "wkh": wk_h, "wkl": wk_l,
                       "wvh": wv_h, "wvl": wv_l})
        else:
            im.update({"ht": hts[b], "wq": wq_h, "wk": wk_h, "wv": wv_h})
        if AT_SPLIT:
            im.update({"woh": wo_h, "wol": wo_l})
        else:
            im.update({"wo": wo_h})
        in_maps.append(im)
        in_maps[-1].update({
            "bq": np.ascontiguousarray(
                (bq[gs * DQ:(gs + 1) * DQ] * QPRE).reshape(NMT, P).T
            ).astype(np.float32),
            "bk": bk[gs * DKV:(gs + 1) * DKV].reshape(P, 1).astype(np.float32),
            "bv": bv[gs * DKV:(gs + 1) * DKV].reshape(P, 1).astype(np.float32),
        })
    return in_maps


def unshard(results, bo):
    bo = np.asarray(bo, np.float32)
    out = np.empty((B, S, HID), np.float32)
    for b in range(B):
        acc = np.zeros((S, HID), np.float64)
        for gs in range(GS):
            acc += results[b * GS + gs]["opart"].astype(np.float32)
        out[b] = (acc + bo).astype(np.float32)
    return out


def kernel(hidden_state, attention_mask, Wq, bq, Wk, bk, Wv, bv, Wo, bo):
    # attention_mask is all-ones for this problem -> identity.
    nc = build_nc()
    in_maps = make_in_maps(hidden_state, Wq, bq, Wk, bk, Wv, bv, Wo)
    res = run_bass_kernel_spmd(nc, in_maps, list(range(NCORES)))
    return unshard(res.results, bo)


# revision 32
# speedup vs baseline: 1.0067x; 1.0030x over previous
"""GroupedQueryAttention Trainium2 kernel (v2).

Sharding: 8 cores = 2 (batch) x 4 (KV-head groups). Each core handles one
batch and 2 KV heads (8 query heads, DQ=512 q dims, DKV=128 kv dims).

Per-core pipeline (CoreSim matmul cost = out_cols x cycles_per_row; bf16 is
1.0, fp8+DoubleRow 0.5 with 2x contraction per instruction):
  - projections: qT (prescaled), k, v
  - QK^T per head into 2-bank psum "duos" [128t, 2, 512s] (bf16)
  - exp split: Activation engine (exact exp) + DVE (exp2 bit-trick)
  - PV in [s, d] orientation (16x fewer streamed cols than [d, s]):
    lhsT = ex duo slice, rhs = v tiles with a ones column -> Z lands in col 64
  - normalize on s-partitions (DVE reciprocal + broadcast mult)
  - DMA-transpose attn [s,d] -> attnT [d,s] (XBAR crossbar, no PE cost)
  - o-proj row-parallel; host sums the 4 partials per batch and adds bo.

PSUM: "sc" tag [128,2,512] x3 slots (6 banks; score duos AND PV half-heads
rotate through it) + "b1" tag [128,512] x2 (proj/o-proj) = 8 banks.
"""

import numpy as np
import ml_dtypes

import concourse.bass as bass
import concourse.mybir as mybir
import concourse.tile as tile
from concourse import bacc
from concourse.bass_utils import run_bass_kernel_spmd

# ---- problem dims ----
P = 128
B, S, HID = 2, 2048, 2048
NH, G = 32, 8
HG = NH // G            # 4 query heads per KV head
D = HID // NH           # 64
NCORES = 8
GS = NCORES // B        # 4 head-group shards
DQ = HID // GS          # 512 q dims per core
DKV = G * D // GS       # 128 kv dims per core (2 KV heads)
CH = 512                # s-chunk width
NCH = S // CH           # 4
KT = HID // P           # 16 contraction tiles (bf16 proj)
NDR = KT // 2           # 8 DoubleRow contraction tiles (fp8 proj)
TT = S // P             # 16 key tiles
NHEADS = 8              # query heads per core
NMT = DQ // P           # 4 q-proj output tiles

# ---- config flags (accuracy-gated) ----
PROJ_SPLIT = True       # two-term fp8 DoubleRow projections (~bf16 accuracy)
W_F8 = False            # fp8 exp weights + fp8 v -> PV DoubleRow
AT_SPLIT = True         # two-term fp8 DoubleRow o-proj
TRICK_PER16 = 0         # duos per 16 routed to DVE exp2 bit-trick (0=Act only)

f32 = mybir.dt.float32
bf16 = mybir.dt.bfloat16
f8 = mybir.dt.float8e4
f8l = mybir.dt.float8e5
i32 = mybir.dt.int32
EXPF = mybir.ActivationFunctionType.Exp
DR = mybir.MatmulPerfMode.DoubleRow
ADD = mybir.AluOpType.add
MULT = mybir.AluOpType.mult

NP_BF16 = ml_dtypes.bfloat16
NP_F8 = ml_dtypes.float8_e4m3
NP_F8L = ml_dtypes.float8_e5m2

SCALE = 1.0 / float(np.sqrt(D))
POW_N = 16384.0                       # act exp scale (scores pre-scaled by SCALE/POW_N)
QPRE = SCALE / POW_N
WSCALE = 2.0 ** -8                    # keeps exp weights under fp8e4m3 max
LNW = float(np.log(WSCALE))
LOG2E = float(np.log2(np.e))
# exp2 bit-trick producing bf16 bit patterns in int16 (single DVE pass):
# i16 = (x*POW_N*log2e*2^23 + (127+log2(WSCALE)-corr)*2^23) / 2^16
TRICK_K = POW_N * LOG2E * (2.0 ** 23) / 65536.0
TRICK_B = float((127.0 + np.log2(WSCALE) - np.log2(1.0443))
                * (2.0 ** 23) / 65536.0)

W_DT = f8 if W_F8 else bf16


def _emit(tc):
    nc = tc.nc

    # ---- DRAM ----
    if PROJ_SPLIT:
        # hi (e4m3) / lo (e5m2) pairs, DoubleRow plane-packed
        ht_dh = nc.dram_tensor("hth", [NCH, NDR, P, 2, CH], f8, kind="ExternalInput")
        ht_dl = nc.dram_tensor("htl", [NCH, NDR, P, 2, CH], f8l, kind="ExternalInput")
        wq_dh = nc.dram_tensor("wqh", [P, NDR, 2, DQ], f8, kind="ExternalInput")
        wq_dl = nc.dram_tensor("wql", [P, NDR, 2, DQ], f8l, kind="ExternalInput")
        wk_dh = nc.dram_tensor("wkh", [P, NDR, 2, DKV], f8, kind="ExternalInput")
        wk_dl = nc.dram_tensor("wkl", [P, NDR, 2, DKV], f8l, kind="ExternalInput")
        wv_dh = nc.dram_tensor("wvh", [P, NDR, 2, DKV], f8, kind="ExternalInput")
        wv_dl = nc.dram_tensor("wvl", [P, NDR, 2, DKV], f8l, kind="ExternalInput")
    else:
        ht_d = nc.dram_tensor("ht", [NCH, KT, P, CH], bf16, kind="ExternalInput")
        wq_d = nc.dram_tensor("wq", [P, KT, DQ], bf16, kind="ExternalInput")
        wk_d = nc.dram_tensor("wk", [P, KT, DKV], bf16, kind="ExternalInput")
        wv_d = nc.dram_tensor("wv", [P, KT, DKV], bf16, kind="ExternalInput")
    if AT_SPLIT:
        wo_dh = nc.dram_tensor("woh", [P, 2, 2, HID], f8, kind="ExternalInput")
        wo_dl = nc.dram_tensor("wol", [P, 2, 2, HID], f8l, kind="ExternalInput")
    else:
        wo_d = nc.dram_tensor("wo", [P, NMT, HID], bf16, kind="ExternalInput")
    bq_d = nc.dram_tensor("bq", [P, NMT], f32, kind="ExternalInput")  # pre x QPRE
    bk_d = nc.dram_tensor("bk", [P, 1], f32, kind="ExternalInput")
    bv_d = nc.dram_tensor("bv", [P, 1], f32, kind="ExternalInput")
    opart = nc.dram_tensor("opart", [S, HID], bf16, kind="ExternalOutput")

    # ---- SBUF pools ----
    consts = tc.alloc_tile_pool(name="consts", bufs=1)
    wpool = tc.alloc_tile_pool(name="wpool", bufs=1)
    htp = tc.alloc_tile_pool(name="htp", bufs=4)
    persist = tc.alloc_tile_pool(name="persist", bufs=1)
    expool = tc.alloc_tile_pool(name="expool", bufs=1)
    work = tc.alloc_tile_pool(name="work", bufs=1)

    bq_t = consts.tile([P, NMT], f32)
    nc.sync.dma_start(out=bq_t[:], in_=bq_d[:])
    bk_t = consts.tile([P, 1], f32)
    nc.sync.dma_start(out=bk_t[:], in_=bk_d[:])
    bv_t = consts.tile([P, 1], f32)
    nc.sync.dma_start(out=bv_t[:], in_=bv_d[:])
    lnw_t = consts.tile([P, 1], f32)
    nc.gpsimd.memset(lnw_t[:], LNW)

    if PROJ_SPLIT:
        wq_sbh = wpool.tile([P, NDR, 2, DQ], f8)
        wq_sbl = wpool.tile([P, NDR, 2, DQ], f8l)
        wk_sbh = wpool.tile([P, NDR, 2, DKV], f8)
        wk_sbl = wpool.tile([P, NDR, 2, DKV], f8l)
        wv_sbh = wpool.tile([P, NDR, 2, DKV], f8)
        wv_sbl = wpool.tile([P, NDR, 2, DKV], f8l)
    else:
        wq_sbh = wpool.tile([P, KT, DQ], bf16)
        wk_sbh = wpool.tile([P, KT, DKV], bf16)
        wv_sbh = wpool.tile([P, KT, DKV], bf16)
    if AT_SPLIT:
        wo_sbh = wpool.tile([P, 2, 2, HID], f8)
        wo_sbl = wpool.tile([P, 2, 2, HID], f8l)
    else:
        wo_sbh = wpool.tile([P, NMT, HID], bf16)

    # persistent activations
    qT_sb = persist.tile([P, NMT, S], bf16)        # prescaled q: [dpair, pair, s]
    ktrepA = persist.tile([P, S], bf16)            # kv head 0 on both halves
    ktrepB = persist.tile([P, S], bf16)            # kv head 1 on both halves
    k_sb = persist.tile([P, S], bf16)
    if W_F8:
        v_dr = persist.tile([P, NDR, 2, 2, 65], f8)   # [t, j, i(plane), g, dv|1]
        nc.gpsimd.memset(v_dr[:, :, :, :, 64:65], 1.0)
    else:
        v_nd = persist.tile([P, TT, 2, 65], bf16)     # [t, tt, g, dv|1]
        nc.gpsimd.memset(v_nd[:, :, :, 64:65], 1.0)
    attn_nrm = [persist.tile([P, 4, NHEADS, D], bf16, name=f"anrm{i}")
                for i in range(2)]

    ht_tiles = {}
    attnT = {}
    state = {"duo": 0, "misc": [], "pv": [], "epi": [], "credit": 0.0, "hold": 0}

    def misc_defer(cost_ns, fn):
        state["misc"].append((cost_ns, fn))

    def drain_misc(credit_ns):
        state["credit"] += credit_ns
        while state["misc"] and state["credit"] > 0:
            cost, fn = state["misc"].pop(0)
            fn()
            state["credit"] -= cost

    def pop_pv():
        if state["hold"]:
            return
        if state["pv"]:
            state["pv"].pop(0)()
        elif state["epi"]:
            state["epi"].pop(0)()

    def flush_all():
        while state["pv"]:
            state["pv"].pop(0)()
        while state["epi"]:
            state["epi"].pop(0)()
        while state["misc"]:
            state["misc"].pop(0)[1]()

    with tc.tile_pool(name="ps_sc", bufs=2, space="PSUM") as ps_sc, \
         tc.tile_pool(name="ps_b1", bufs=2, space="PSUM") as ps_b1:

        def load_ht(c, splits=1, eng=None):
            eng = eng or nc.sync
            if PROJ_SPLIT:
                hth = htp.tile([P, NDR, 2, CH], f8, tag="hth", name=f"hth{c}")
                eng.dma_start(out=hth[:],
                              in_=ht_dh[c].rearrange("kt p i s -> p kt i s"))
                htl = htp.tile([P, NDR, 2, CH], f8l, tag="htl", name=f"htl{c}")
                eng.dma_start(out=htl[:],
                              in_=ht_dl[c].rearrange("kt p i s -> p kt i s"))
                ht_tiles[c] = (hth, htl)
            else:
                htt = htp.tile([P, KT, CH], bf16, tag="hth", name=f"ht{c}")
                hsrc = ht_d[c].rearrange("kt p s -> p kt s")
                step = KT // splits
                for s0 in range(0, KT, step):
                    eng.dma_start(out=htt[:, s0:s0 + step],
                                  in_=hsrc[:, s0:s0 + step])
                ht_tiles[c] = htt

        def proj_mm(out_ap, w_h, w_l, mcols, c):
            if PROJ_SPLIT:
                hth, htl = ht_tiles[c]
                terms = [(w_h, hth), (w_h, htl), (w_l, hth)]
                for ti, (wt, ht_t) in enumerate(terms):
                    for kt in range(NDR):
                        nc.tensor.matmul(out_ap, wt[:, kt, :, mcols],
                                         ht_t[:, kt, :, :],
                                         start=(ti == 0 and kt == 0),
                                         stop=(ti == 2 and kt == NDR - 1),
                                         perf_mode=DR)
            else:
                htt = ht_tiles[c]
                for kt in range(KT):
                    nc.tensor.matmul(out_ap, w_h[:, kt, mcols], htt[:, kt, :],
                                     start=(kt == 0), stop=(kt == KT - 1))

        def emit_q_proj_mt(c, mt):
            cs = slice(c * CH, (c + 1) * CH)
            b1 = ps_b1.tile([P, 2, CH], f32, tag="pv2", name=f"qp{c}_{mt}")
            proj_mm(b1[:, 0, :], wq_sbh, wq_sbl if PROJ_SPLIT else None,
                    slice(mt * P, (mt + 1) * P), c)
            nc.vector.tensor_scalar(out=qT_sb[:, mt, cs], in0=b1[:, 0, :],
                                    scalar1=QPRE, scalar2=bq_t[:, mt:mt + 1],
                                    op0=MULT, op1=ADD)

        def defer_q_proj_split(c, mt):
            # three deferred sub-items sharing one psum tile; pv2-tag pops are
            # held off between them so the slot ring can't rotate mid-group
            box = {}

            def sub(term):
                def go():
                    if term == 0:
                        state["hold"] += 1
                        box["b1"] = ps_b1.tile([P, 2, CH], f32, tag="pv2",
                                               name=f"qp{c}_{mt}")
                    hth, htl = ht_tiles[c]
                    wt, ht_t = [(wq_sbh, hth), (wq_sbh, htl), (wq_sbl, hth)][term]
                    for kt in range(NDR):
                        nc.tensor.matmul(box["b1"][:, 0, :],
                                         wt[:, kt, :, mt * P:(mt + 1) * P],
                                         ht_t[:, kt, :, :],
                                         start=(term == 0 and kt == 0),
                                         stop=(term == 2 and kt == NDR - 1),
                                         perf_mode=DR)
                    if term == 2:
                        cs = slice(c * CH, (c + 1) * CH)
                        nc.vector.tensor_scalar(
                            out=qT_sb[:, mt, cs], in0=box["b1"][:, 0, :],
                            scalar1=QPRE, scalar2=bq_t[:, mt:mt + 1],
                            op0=MULT, op1=ADD)
                        state["hold"] -= 1
                return go
            for t in range(3):
                misc_defer(900, sub(t))

        def emit_k_proj(c):
            cs = slice(c * CH, (c + 1) * CH)
            b1 = ps_b1.tile([P, 2, CH], f32, tag="pv2", name=f"kp{c}")
            proj_mm(b1[:, 0, :], wk_sbh, wk_sbl if PROJ_SPLIT else None,
                    slice(0, DKV), c)
            nc.vector.tensor_scalar_add(k_sb[:, cs], b1[:, 0, :], bk_t[:, 0:1])
            # duplicate each kv head onto both partition halves for paired QK
            nc.sync.dma_start(out=ktrepA[0:D, cs], in_=k_sb[0:D, cs])
            nc.sync.dma_start(out=ktrepA[D:P, cs], in_=k_sb[0:D, cs])
            nc.sync.dma_start(out=ktrepB[0:D, cs], in_=k_sb[D:P, cs])
            nc.sync.dma_start(out=ktrepB[D:P, cs], in_=k_sb[D:P, cs])

        def emit_v_proj(c):
            b1 = ps_b1.tile([P, 2, CH], f32, tag="pv2", name=f"vp{c}")
            proj_mm(b1[:, 0, :], wv_sbh, wv_sbl if PROJ_SPLIT else None,
                    slice(0, DKV), c)
            vstage = work.tile([P, CH], bf16, tag="vstage", bufs=2, name=f"vs{c}")
            nc.vector.tensor_scalar_add(vstage[:], b1[:, 0, :], bv_t[:, 0:1])
            vtr = work.tile([P, 4, P], bf16, tag="vtr", bufs=2, name=f"vtr{c}")
            nc.sync.dma_start_transpose(vtr[:], vstage[:])   # [t, tt, dkv]
            for g in range(2):
                gsl = slice(g * D, (g + 1) * D)
                if W_F8:
                    for jj in range(2):
                        j = 2 * c + jj
                        nc.vector.tensor_copy(v_dr[:, j, :, g, 0:D],
                                              vtr[:, 2 * jj:2 * jj + 2, gsl])
                else:
                    nc.vector.tensor_copy(v_nd[:, 4 * c:4 * (c + 1), g, 0:D],
                                          vtr[:, :, gsl])

        def emit_exp(duo, ex):
            # ex is an int16-backed tile; write bf16 BITS either via the Act
            # exp (bitcast view) or the DVE exp2 bit-trick (int16 value cast)
            i = state["duo"]
            state["duo"] += 1
            if (i % 16) < TRICK_PER16:
                nc.vector.tensor_scalar(out=ex[:], in0=duo[:], scalar1=TRICK_K,
                                        scalar2=TRICK_B, op0=MULT, op1=ADD)
            else:
                nc.scalar.activation(out=ex[:].bitcast(W_DT), in_=duo[:],
                                     func=EXPF, scale=POW_N, bias=lnw_t[:])

        def emit_pv_half(c, h, half, ex_tiles):
            g = h // 4
            pv = ps_b1.tile([P, 2, CH], f32, tag="pv2", name=f"pv{c}_{h}_{half}")
            for sl in range(2):
                st = 2 * half + sl
                ss = slice(st * P, (st + 1) * P)
                if W_F8:
                    for j in range(NDR):
                        nc.tensor.matmul(pv[:, sl, 0:65], ex_tiles[j][:, :, ss],
                                         v_dr[:, j, :, g, :],
                                         start=(j == 0), stop=(j == NDR - 1),
                                         perf_mode=DR)
                else:
                    for t in range(TT):
                        nc.tensor.matmul(pv[:, sl, 0:65],
                                         ex_tiles[t // 2][:, t % 2, ss],
                                         v_nd[:, t, g, :],
                                         start=(t == 0), stop=(t == TT - 1))
            zr = work.tile([P, 2, 1], f32, tag="zr", bufs=3, name="zr")
            nc.vector.reciprocal(zr[:], pv[:, :, 64:65])
            nc.vector.tensor_tensor(
                out=attn_nrm[c % 2][:, 2 * half:2 * half + 2, h, :],
                in0=pv[:, :, 0:D], in1=zr[:].broadcast_to((P, 2, D)), op=MULT)

        def emit_attnT(c, st):
            at = work.tile([P, NMT, P], bf16, tag="attnT", bufs=6,
                           name=f"at{c}_{st}")
            nc.sync.dma_start_transpose(at[:], attn_nrm[c % 2][:, st, :, :])
            if AT_SPLIT:
                ath = work.tile([P, NMT, P], f8, tag="attnTh", bufs=10,
                                name=f"ath{c}_{st}")
                nc.gpsimd.tensor_copy(ath[:], at[:])
                atl = work.tile([P, NMT, P], f8l, tag="attnTl", bufs=10,
                                name=f"atl{c}_{st}")
                nc.vector.tensor_tensor(out=atl[:], in0=at[:], in1=ath[:],
                                        op=mybir.AluOpType.subtract)
                attnT[(c, st)] = (ath, atl)
            else:
                attnT[(c, st)] = at

        def emit_oproj(c, st, hc):
            at = attnT.pop((c, st)) if hc == NMT - 1 else attnT[(c, st)]
            ss = slice((c * 4 + st) * P, (c * 4 + st + 1) * P)
            hs = slice(hc * CH, (hc + 1) * CH)
            b1 = ps_b1.tile([P, 2, CH], f32, tag="pv2", name=f"op{c}_{st}_{hc}")
            if AT_SPLIT:
                ath, atl = at
                terms = [(ath, wo_sbh), (ath, wo_sbl), (atl, wo_sbh)]
                for ti, (att, wot) in enumerate(terms):
                    for j in range(2):
                        nc.tensor.matmul(b1[:, 0, :], att[:, 2 * j:2 * j + 2, :],
                                         wot[:, j, :, hs],
                                         start=(ti == 0 and j == 0),
                                         stop=(ti == 2 and j == 1), perf_mode=DR)
            else:
                for kt in range(NMT):
                    nc.tensor.matmul(b1[:, 0, :], at[:, kt, :], wo_sbh[:, kt, hs],
                                     start=(kt == 0), stop=(kt == NMT - 1))
            ostg = work.tile([P, CH], bf16, tag="ostg", bufs=4, name="ostg")
            if c == NCH - 1:
                nc.scalar.copy(ostg[:], b1[:, 0, :])
            else:
                nc.vector.tensor_copy(ostg[:], b1[:, 0, :])
            nc.sync.dma_start(out=opart[ss, hs], in_=ostg[:])

        # ---------- prologue ----------
        if PROJ_SPLIT:
            nc.scalar.dma_start(out=wk_sbh[:], in_=wk_dh[:])
            nc.scalar.dma_start(out=wk_sbl[:], in_=wk_dl[:])
            nc.scalar.dma_start(out=wv_sbh[:], in_=wv_dh[:])
            nc.scalar.dma_start(out=wv_sbl[:], in_=wv_dl[:])
        else:
            nc.scalar.dma_start(out=wk_sbh[:], in_=wk_d[:])
            nc.scalar.dma_start(out=wv_sbh[:], in_=wv_d[:])
        load_ht(0)
        load_ht(1, eng=nc.scalar)
        if PROJ_SPLIT:
            nc.scalar.dma_start(out=wq_sbh[:], in_=wq_dh[:])
            nc.scalar.dma_start(out=wq_sbl[:], in_=wq_dl[:])
        else:
            nc.scalar.dma_start(out=wq_sbh[:], in_=wq_d[:])
        load_ht(2)
        load_ht(3)
        if AT_SPLIT:
            nc.sync.dma_start(out=wo_sbh[:], in_=wo_dh[:])
            nc.sync.dma_start(out=wo_sbl[:], in_=wo_dl[:])
        else:
            nc.sync.dma_start(out=wo_sbh[:], in_=wo_d[:])
        for c in range(NCH):
            emit_k_proj(c)
        emit_q_proj_mt(0, 0)
        for c in range(NCH):
            misc_defer(3400, (lambda cc: lambda: emit_v_proj(cc))(c))
        for mt in range(1, NMT):
            misc_defer(3400, (lambda m: lambda: emit_q_proj_mt(0, m))(mt))
        for c in (1, 2, 3):
            for mt in range(NMT):
                misc_defer(3400, (lambda cc, m: lambda: emit_q_proj_mt(cc, m))(c, mt))

        # ---------- main loop ----------
        for c in range(NCH):
            cs = slice(c * CH, (c + 1) * CH)
            for h in range(NHEADS):
                pair, e = h // 2, h % 2
                ktrep = ktrepA if h < 4 else ktrepB
                erange = slice(e * D, (e + 1) * D)
                ex_tiles = []
                for j2 in range(NDR):
                    duo = ps_sc.tile([P, 2, CH], f32, tag="sc", name="duo")
                    for i2 in range(2):
                        ts_ = slice((2 * j2 + i2) * P, (2 * j2 + i2 + 1) * P)
                        nc.tensor.matmul(duo[:, i2, :], ktrep[erange, ts_],
                                         qT_sb[erange, pair, cs],
                                         tile_position=(e * D, 0),
                                         start=True, stop=True)
                    ex = expool.tile([P, 2, CH], mybir.dt.int16, tag="ex",
                                     bufs=12, name="ex")
                    emit_exp(duo, ex)
                    ex_tiles.append(ex[:].bitcast(W_DT))
                    pop_pv()
                    # chunk-0 head 0/1: force v + q0 projections through before
                    # the first PV pop needs them
                    drain_misc(3400 if (c == 0 and h < 2) else 430)
                for half in range(2):
                    state["pv"].append(
                        (lambda cc, hh, hf, exs:
                         lambda: emit_pv_half(cc, hh, hf, exs))(c, h, half, ex_tiles))
            # chunk epilogue: pops only when the PV queue is empty, which
            # keeps attnT after this chunk's last PV halves
            for st in range(4):
                state["epi"].append(
                    (lambda cc, s_: lambda: emit_attnT(cc, s_))(c, st))
            for st in range(4):
                for hc in range(NMT):
                    state["epi"].append(
                        (lambda cc, s_, hh: lambda: emit_oproj(cc, s_, hh))
                        (c, st, hc))
        flush_all()

    for pool in (work, expool, persist, htp, wpool, consts):
        pool.release()


_NC_CACHE = None


def build_nc():
    global _NC_CACHE
    if _NC_CACHE is None:
        nc = bacc.Bacc("TRN2")
        with tile.TileContext(nc) as tc:
            _emit(tc)
        nc.compile()
        _NC_CACHE = nc
    return _NC_CACHE


def _split_f8(x):
    x = np.asarray(x, np.float32)
    hi = x.astype(NP_F8)
    lo = (x - hi.astype(np.float32)).astype(NP_F8L)
    return hi, lo


def _pack_dr_w(Wslice):
    # Wslice [M, HID] -> ([P, NDR, 2, M] e4m3 hi, same-shape e5m2 lo)
    M = Wslice.shape[0]
    w = Wslice.T.reshape(NDR, 2, P, M).transpose(2, 0, 1, 3)   # [p, kt, i, m]
    hi, lo = _split_f8(w)
    return np.ascontiguousarray(hi), np.ascontiguousarray(lo)


def make_in_maps(hidden_state, Wq, bq, Wk, bk, Wv, bv, Wo):
    hidden_state = np.asarray(hidden_state, np.float32)
    Wq, Wk, Wv, Wo = (np.asarray(a, np.float32) for a in (Wq, Wk, Wv, Wo))
    bq, bk, bv = (np.asarray(a, np.float32) for a in (bq, bk, bv))

    hts = []
    for b in range(B):
        htb = hidden_state[b].T                  # [HID, S]
        if PROJ_SPLIT:
            h4 = htb.reshape(NDR, 2, P, NCH, CH).transpose(3, 0, 2, 1, 4)
            hi, lo = _split_f8(h4)               # [c, kt, p, i, s]
            hts.append((np.ascontiguousarray(hi), np.ascontiguousarray(lo)))
        else:
            h4 = htb.reshape(KT, P, NCH, CH)
            hts.append(np.ascontiguousarray(
                h4.transpose(2, 0, 1, 3)).astype(NP_BF16))

    in_maps = []
    for core in range(NCORES):
        b, gs = divmod(core, GS)
        wq_s = Wq[gs * DQ:(gs + 1) * DQ, :]       # [DQ, HID]
        wk_s = Wk[gs * DKV:(gs + 1) * DKV, :]
        wv_s = Wv[gs * DKV:(gs + 1) * DKV, :]
        wo_s = Wo[:, gs * DQ:(gs + 1) * DQ]       # [HID, DQ]
        if PROJ_SPLIT:
            wq_h, wq_l = _pack_dr_w(wq_s)
            wk_h, wk_l = _pack_dr_w(wk_s)
            wv_h, wv_l = _pack_dr_w(wv_s)
        else:
            wq_h = np.ascontiguousarray(
                wq_s.T.reshape(KT, P, DQ).transpose(1, 0, 2)).astype(NP_BF16)
            wk_h = np.ascontiguousarray(
                wk_s.T.reshape(KT, P, DKV).transpose(1, 0, 2)).astype(NP_BF16)
            wv_h = np.ascontiguousarray(
                wv_s.T.reshape(KT, P, DKV).transpose(1, 0, 2)).astype(NP_BF16)
        if AT_SPLIT:
            wot = wo_s.T.reshape(2, 2, P, HID).transpose(2, 0, 1, 3)  # [p,j,i,h]
            wo_h, wo_l = _split_f8(wot)
            wo_h, wo_l = np.ascontiguousarray(wo_h), np.ascontiguousarray(wo_l)
        else:
            wo_h = np.ascontiguousarray(
                wo_s.T.reshape(NMT, P, HID).transpose(1, 0, 2)).astype(NP_BF16)
        im = {}
        if PROJ_SPLIT:
            im.update({"hth": hts[b][0], "htl": hts[b][1],
                       "wqh": wq_h, "wql": wq_l, "wkh": wk_h, "wkl": wk_l,
                       "wvh": wv_h, "wvl": wv_l})
        else:
            im.update({"ht": hts[b], "wq": wq_h, "wk": wk_h, "wv": wv_h})
        if AT_SPLIT:
            im.update({"woh": wo_h, "wol": wo_l})
        else:
            im.update({"wo": wo_h})
        in_maps.append(im)
        in_maps[-1].update({
            "bq": np.ascontiguousarray(
                (bq[gs * DQ:(gs + 1) * DQ] * QPRE).reshape(NMT, P).T
            ).astype(np.float32),
            "bk": bk[gs * DKV:(gs + 1) * DKV].reshape(P, 1).astype(np.float32),
            "bv": bv[gs * DKV:(gs + 1) * DKV].reshape(P, 1).astype(np.float32),
        })
    return in_maps


def unshard(results, bo):
    bo = np.asarray(bo, np.float32)
    out = np.empty((B, S, HID), np.float32)
    for b in range(B):
        acc = np.zeros((S, HID), np.float64)
        for gs in range(GS):
            acc += results[b * GS + gs]["opart"].astype(np.float32)
        out[b] = (acc + bo).astype(np.float32)
    return out


def kernel(hidden_state, attention_mask, Wq, bq, Wk, bk, Wv, bv, Wo, bo):
    # attention_mask is all-ones for this problem -> identity.
    nc = build_nc()
    in_maps = make_in_maps(hidden_state, Wq, bq, Wk, bk, Wv, bv, Wo)
    res = run_bass_kernel_spmd(nc, in_maps, list(range(NCORES)))
    return unshard(res.results, bo)
